# revision 34
# baseline (speedup 1.0000x reference)
"""Cross-attention (B=2, Q=1024, N=4096, C=768, H=12, D=64) with bilinearly
interpolated relative position bias, on 8 Trainium2 NeuronCores.

Sharding: core c handles batch b = c//4 and heads 3*(c%4) .. 3*(c%4)+2
(tensor-parallel over heads, data-parallel over batch). Each core outputs, per
head, the unnormalized attention output projected through Wo_h, plus the
softmax denominators; the host divides, sums the partials, and adds bo.

Device algorithm per core (fp16 matmul operands, fp32 accumulation):
  qbT[h]  = (Wq_h^T @ q^T) * scale + bq          [64, 1024]   (d-major)
  kbT[h]  = Wk_h^T @ kv^T + bk                   [64, 4096]
  vb[n]   = kv @ Wv_h + bv                       [4096, 64]   (n-major)
  S^T     = [kbT; Wn]^T-contraction [qbT; B1T]   K=96 fuses the interpolated
            bias: bias[h,q,n] = sum_j B1[h,q,j] * Wn[j,n]
  E^T     = exp(S^T)            (no max-subtraction; logits are O(1))
  O^T[h]  = [vb_h | 1]^T @ E^T                   [65, 1024]  row 64 = denom
  F[h]    = O^T[h]^T-contraction Wo_h            [1024, 768]  (unnormalized)

Schedule: pass1 = heads 0+1 S/exp interleaved with the streamed k/v
projections (O accumulated for h0 only; h1's E tiles stored in SBUF), pass2 =
h2 attention + h1's O from the store + Wo(h0,h1) interleaved, tail = Wo(h2).
This keeps the scalar engine (exp) saturated during the projection-heavy pass
and the PE saturated during the exp-heavy pass. DMA triggers cost ~0.7us each
on the SP sequencer, so inputs are packed into few large transfers and the
constant rows (interp weights, bias rows, denominator ones) are written with
one DMA / one ALU op instead of per-tile transfers.
"""

import numpy as np

B, Q, N, C = 2, 1024, 4096, 768
H, D, REL = 12, 64, 32
SCALE = 1.0 / np.sqrt(D)
HPC = 3            # heads per core
N_CORES = 8
NCH = 8            # 512-wide n-chunks
NCI = N // 128

_COMPILED = None   # cached nc across kernel() calls


def _lin_coords(n_out, n_in):
    pos = np.arange(n_out, dtype=np.float32) * np.float32((n_in - 1) / (n_out - 1))
    lo = np.clip(np.floor(pos).astype(np.int32), 0, n_in - 1)
    hi = np.clip(lo + 1, 0, n_in - 1)
    w = (pos - lo.astype(np.float32)).astype(np.float32)
    return lo, hi, w


def _host_bias_parts(rel_pos_bias):
    """B1: [H, Q, 32] q-interpolated bias; Wn: [32, N] n-interp weights."""
    lq, hq, wq = _lin_coords(Q, REL)
    ln, hn, wn = _lin_coords(N, REL)
    b1 = (rel_pos_bias[:, lq, :] * (1.0 - wq)[None, :, None]
          + rel_pos_bias[:, hq, :] * wq[None, :, None]).astype(np.float32)
    w_n = np.zeros((REL, N), np.float32)
    np.add.at(w_n, (ln, np.arange(N)), (1.0 - wn))
    np.add.at(w_n, (hn, np.arange(N)), wn)
    return b1, w_n


def _build():
    import concourse.tile as tile
    from concourse import bacc, mybir
    import concourse.bass as bass

    F32 = mybir.dt.float32
    F16 = mybir.dt.float16
    KT = 6  # C // 128 contraction tiles

    nc = bacc.Bacc("TRN2", target_bir_lowering=False, debug=False,
                   enable_asserts=False, num_devices=N_CORES)

    qT = nc.dram_tensor("qT", [128, 2, KT, 512], F16, kind="ExternalInput")
    kvT = nc.dram_tensor("kvT", [128, NCH, KT, 512], F16,
                         kind="ExternalInput")
    wqp = nc.dram_tensor("wqp", [128, KT, 192], F16, kind="ExternalInput")
    wkv = nc.dram_tensor("wkv", [128, KT, 384], F16, kind="ExternalInput")
    wo = nc.dram_tensor("wo", [D, HPC, C], F16, kind="ExternalInput")
    bq6 = nc.dram_tensor("bq6", [D, 2 * HPC], F32, kind="ExternalInput")
    bvb = nc.dram_tensor("bvb", [128, 192], F32, kind="ExternalInput")  # bcast bv
    b1t = nc.dram_tensor("b1t", [REL, HPC, Q], F16, kind="ExternalInput")
    w_n3 = nc.dram_tensor("w_n3", [REL, HPC, N], F16, kind="ExternalInput")
    out_p = nc.dram_tensor("out_p", [HPC, Q, C], F16, kind="ExternalOutput")
    den = nc.dram_tensor("den", [HPC, Q], F32, kind="ExternalOutput")

    EXP = mybir.ActivationFunctionType.Exp
    ADD = mybir.AluOpType.add
    MULT = mybir.AluOpType.mult

    with tile.TileContext(nc) as tc:
        with (
            tc.tile_pool(name="wpool", bufs=1) as wpool,
            tc.tile_pool(name="persist", bufs=1) as pers,
            tc.tile_pool(name="stream", bufs=2) as stream,
            tc.tile_pool(name="pexp", bufs=3) as pexp,
            tc.tile_pool(name="tailp", bufs=2) as tailp,
        ):
            # ---- weights: wq first (unblocks Q-proj), wo deferred ----
            wq_sb = wpool.tile([128, KT, 192], F16, name="wq_sb")
            nc.sync.dma_start(out=wq_sb, in_=wqp[:, :, :])
            bq6_sb = wpool.tile([D, 2 * HPC], F32, name="bq6_sb")
            nc.sync.dma_start(out=bq6_sb, in_=bq6[:, :])
            bqs_sb = bq6_sb[:, 0:HPC]
            bks_sb = bq6_sb[:, HPC:2 * HPC]

            # ---- persistent per-head tiles ----
            qTp = pers.tile([96, HPC, Q], F16, name="qTp", tag="qTp")
            kbT = pers.tile([96, HPC, NCH, 512], F16, name="kbT", tag="kbT")
            vb = pers.tile([128, NCI, HPC, 65], F16, name="vb", tag="vb")
            px1s = pers.tile([128, NCI, Q], F16, name="px1s", tag="px1s")

            # ---- phase A: q projection (qT loaded in contiguous halves) ----
            def load_kv(ch):
                kvc = stream.tile([128, KT, 512], F16, name="kvc", tag="kvc")
                nc.sync.dma_start(out=kvc, in_=kvT[:, ch])
                return kvc

            P = {}  # current transient-PSUM pool, swapped per pass

            with tc.tile_pool(name="qload", bufs=1) as qload, \
                 tc.tile_pool(name="psA", bufs=1, space="PSUM") as psA:
                qT_sb = qload.tile([128, 2, KT, 512], F16, name="qT_sb")
                nc.sync.dma_start(out=qT_sb[:, 0], in_=qT[:, 0])
                wkv_sb = wpool.tile([128, KT, 384], F16, name="wkv_sb")
                nc.sync.dma_start(out=wkv_sb, in_=wkv[:, :, :])
                wk_sb = wkv_sb[:, :, 0:192]
                wv_sb = wkv_sb[:, :, 192:384]
                # prefetch kv chunks 0/1 behind the Q-proj-critical loads
                kvc_pre = [load_kv(0), load_kv(1)]
                nc.sync.dma_start(out=qT_sb[:, 1], in_=qT[:, 1])
                bvb_sb = wpool.tile([128, 192], F32, name="bvb_sb")
                nc.sync.dma_start(out=bvb_sb, in_=bvb[:, :])
                nc.sync.dma_start(
                    out=kbT[64:96, :, :, :],
                    in_=w_n3.rearrange("p h (c n) -> p h c n", n=512))
                nc.sync.dma_start(out=qTp[64:96, :, :], in_=b1t[:, :, :])
                # denominator ones-rows of vb: single memset, no DMA
                nc.gpsimd.memset(vb[:, :, :, 64], 1.0)
                for qc in range(2):
                    sl = slice(512 * qc, 512 * qc + 512)
                    psqA = psA.tile([128, 512], F32, name="psqA", tag="psqA")
                    psqB = psA.tile([64, 512], F32, name="psqB", tag="psqB")
                    for t in range(KT):
                        nc.tensor.matmul(psqA, wq_sb[:, t, 0:128],
                                         qT_sb[:, qc, t, :],
                                         start=(t == 0), stop=(t == KT - 1))
                    for t in range(KT):
                        nc.tensor.matmul(psqB, wq_sb[:, t, 128:192],
                                         qT_sb[:, qc, t, :],
                                         start=(t == 0), stop=(t == KT - 1))
                    nc.vector.tensor_scalar_add(qTp[0:64, 0, sl], psqA[0:64, :],
                                                bqs_sb[:, 0:1])
                    nc.vector.tensor_scalar_add(qTp[0:64, 1, sl], psqA[64:128, :],
                                                bqs_sb[:, 1:2])
                    nc.vector.tensor_scalar_add(qTp[0:64, 2, sl], psqB[0:64, :],
                                                bqs_sb[:, 2:3])

            # ---- phase B chunk: k/v projections for one 512-wide n-chunk ----
            def emit_b_chunk(ch, kvc):
                pskA = P['s'].tile([128, 512], F32, name="pskA", tag="psS")
                for t in range(KT):
                    nc.tensor.matmul(pskA, wk_sb[:, t, 0:128], kvc[:, t, :],
                                     start=(t == 0), stop=(t == KT - 1))
                nc.vector.tensor_scalar_add(kbT[0:64, 0, ch, :], pskA[0:64, :],
                                            bks_sb[:, 0:1])
                nc.vector.tensor_scalar_add(kbT[0:64, 1, ch, :], pskA[64:128, :],
                                            bks_sb[:, 1:2])
                pskB = P['s'].tile([64, 512], F32, name="pskB", tag="psS")
                for t in range(KT):
                    nc.tensor.matmul(pskB, wk_sb[:, t, 128:192], kvc[:, t, :],
                                     start=(t == 0), stop=(t == KT - 1))
                nc.vector.tensor_scalar_add(kbT[0:64, 2, ch, :], pskB[0:64, :],
                                            bks_sb[:, 2:3])
                for s in range(4):
                    n128 = 4 * ch + s
                    psv = P['s'].tile([128, 192], F32, name="psv", tag="psS")
                    for t in range(KT):
                        nc.tensor.matmul(psv, kvc[:, t, 128 * s:128 * s + 128],
                                         wv_sb[:, t, :],
                                         start=(t == 0), stop=(t == KT - 1))
                    nc.vector.tensor_tensor(
                        out=vb[:, n128, :, 0:64],
                        in0=psv.rearrange("p (h d) -> p h d", d=64),
                        in1=bvb_sb.rearrange("p (h d) -> p h d", d=64),
                        op=ADD)

            # ---- attention passes ----
            import contextlib

            def emit_s(h, ci):
                c512, s = ci // 4, ci % 4
                ssl = slice(128 * s, 128 * s + 128)
                psS = P['s'].tile([128, Q], F32, name="psS", tag="psS")
                nc.tensor.matmul(psS[:, 0:512], kbT[:, h, c512, ssl],
                                 qTp[:, h, 0:512], start=True, stop=True)
                nc.tensor.matmul(psS[:, 512:1024], kbT[:, h, c512, ssl],
                                 qTp[:, h, 512:1024], start=True, stop=True)
                return psS

            def emit_o(po, h, ci, src):
                st = (ci == 0)
                sp = (ci == NCI - 1)
                nc.tensor.matmul(po[:, 0:512], vb[:, ci, h, :],
                                 src[:, 0:512], start=st, stop=sp)
                nc.tensor.matmul(po[:, 512:1024], vb[:, ci, h, :],
                                 src[:, 512:1024], start=st, stop=sp)

            def emit_fin(po, h):
                onT = tailp.tile([64, Q], F16, name=f"onT{h}", tag=f"onT{h}",
                                 bufs=1)
                nc.vector.tensor_copy(out=onT, in_=po[0:64, :])
                dsb = tailp.tile([1, Q], F32, name=f"dsb{h}", tag="dsb", bufs=2)
                nc.scalar.copy(out=dsb, in_=po[64:65, :])
                nc.sync.dma_start(out=den[h, :], in_=dsb)
                return onT

            def wo_half(psF, h, onT, qt, osb, tail=False):
                tsl = slice(128 * qt, 128 * qt + 128)
                for nc2 in range(2):
                    nsl = slice(384 * nc2, 384 * nc2 + 384)
                    psf = psF.tile([128, 384], F32, name="psf", tag="psf")
                    nc.tensor.matmul(psf, onT[:, tsl], wo_sb[:, h, nsl],
                                     start=True, stop=True)
                    # in-pass copies stay on DVE (scalar is exp-bound there);
                    # tail copies alternate DVE/scalar (both engines idle)
                    if tail and nc2 == 1:
                        nc.scalar.copy(out=osb[:, qt % 2, nsl], in_=psf)
                    else:
                        nc.vector.tensor_copy(out=osb[:, qt % 2, nsl], in_=psf)
                if qt % 2 == 1:
                    nc.sync.dma_start(
                        out=out_p[h, 256 * (qt // 2):256 * (qt // 2) + 256, :]
                            .rearrange("(a p) c -> p a c", p=128),
                        in_=osb)

            # ---- pass 1: h0 + h1 (S/exp), O for h0; b-chunk projections ----
            with contextlib.ExitStack() as pstk:
                psO0 = pstk.enter_context(
                    tc.tile_pool(name="psO0", bufs=1, space="PSUM"))
                po0 = psO0.tile([65, Q], F32, name="po0", tag="po")
                P['s'] = pstk.enter_context(
                    tc.tile_pool(name="psP1", bufs=3, space="PSUM"))
                emit_b_chunk(0, kvc_pre[0])
                psS0_cur = emit_s(0, 0)
                psS1_cur = emit_s(1, 0)
                for ci in range(NCI):
                    if ci + 1 < NCI:
                        if (ci + 1) % 4 == 0:
                            ch = (ci + 1) // 4
                            if ch + 1 < NCH:
                                kvc_pre.append(load_kv(ch + 1))
                            emit_b_chunk(ch, kvc_pre[ch])
                        psS0_nxt = emit_s(0, ci + 1)
                        psS1_nxt = emit_s(1, ci + 1)
                    px0 = pexp.tile([128, Q], F16, name="px0", tag="px")
                    nc.scalar.activation(out=px0, in_=psS0_cur, func=EXP)
                    nc.scalar.activation(out=px1s[:, ci, :], in_=psS1_cur,
                                         func=EXP)
                    emit_o(po0, 0, ci, px0)
                    if ci + 1 < NCI:
                        psS0_cur = psS0_nxt
                        psS1_cur = psS1_nxt
                onT0 = emit_fin(po0, 0)

            # wo loaded between passes (SP queue idle here)
            wo_sb = wpool.tile([D, HPC, C], F16, name="wo_sb")
            nc.sync.dma_start(out=wo_sb, in_=wo[:, :, :])

            # ---- pass 2: h2 attention + h1's O + Wo(h0, h1) ----
            with contextlib.ExitStack() as pstk:
                psO2 = pstk.enter_context(
                    tc.tile_pool(name="psO2", bufs=1, space="PSUM"))
                po2 = psO2.tile([65, Q], F32, name="po2", tag="po")
                P['s'] = pstk.enter_context(
                    tc.tile_pool(name="psP2", bufs=2, space="PSUM"))
                psS2_cur = emit_s(2, 0)
                onT1 = None
                psF = None
                osb = None
                with contextlib.ExitStack() as pstk1:
                    psO1 = pstk1.enter_context(
                        tc.tile_pool(name="psO1", bufs=1, space="PSUM"))
                    po1 = psO1.tile([65, Q], F32, name="po1", tag="po")
                    for ci in range(NCI):
                        if ci + 1 < NCI:
                            psS2_nxt = emit_s(2, ci + 1)
                        px2 = pexp.tile([128, Q], F16, name="px2", tag="px")
                        nc.scalar.activation(out=px2, in_=psS2_cur, func=EXP)
                        emit_o(po2, 2, ci, px2)
                        if ci < 16:
                            emit_o(po1, 1, 2 * ci, px1s[:, 2 * ci, :])
                            emit_o(po1, 1, 2 * ci + 1, px1s[:, 2 * ci + 1, :])
                            if ci == 15:
                                onT1 = emit_fin(po1, 1)
                                pstk1.close()
                                psF = pstk.enter_context(
                                    tc.tile_pool(name="psF", bufs=2,
                                                 space="PSUM"))
                        else:
                            qt = ci - 16
                            hh, qtt = (0, qt) if qt < 8 else (1, qt - 8)
                            if qtt % 2 == 0:
                                osb = tailp.tile([128, 2, C], F16, name="osb",
                                                 tag="osb", bufs=2)
                            wo_half(psF, hh, onT0 if hh == 0 else onT1,
                                    qtt, osb)
                        if ci + 1 < NCI:
                            psS2_cur = psS2_nxt
                onT2 = emit_fin(po2, 2)
            with tc.tile_pool(name="psFt", bufs=3, space="PSUM") as psFt:
                for qt in range(8):
                    if qt % 2 == 0:
                        osb = tailp.tile([128, 2, C], F16, name="osb",
                                         tag="osb", bufs=2)
                    wo_half(psFt, 2, onT2, qt, osb, tail=True)
    nc.compile()
    return nc


def _get_compiled():
    global _COMPILED
    if _COMPILED is None:
        _COMPILED = _build()
    return _COMPILED


def _to_p128(a):
    """[768, M] -> [128, 6, M] partition-major fp16."""
    return np.ascontiguousarray(
        a.reshape(6, 128, -1).transpose(1, 0, 2)).astype(np.float16)


def _chunked(a, w):
    """[128, 6, M] -> [128, M//w, 6, w] (w-wide column chunks contiguous)."""
    p, t, m = a.shape
    return np.ascontiguousarray(
        a.reshape(p, t, m // w, w).transpose(0, 2, 1, 3))


def _make_in_maps(query, key_value, Wq, bq, Wk, bk, Wv, bv, Wo, rel_pos_bias):
    b1, w_n = _host_bias_parts(rel_pos_bias)
    scale = np.float32(SCALE)
    f16 = np.float16
    qTs = [_chunked(_to_p128(np.ascontiguousarray(query[b].T)), 512)
           for b in range(B)]
    kvTs = [_chunked(_to_p128(np.ascontiguousarray(key_value[b].T)), 512)
            for b in range(B)]
    w_n3 = np.ascontiguousarray(
        np.broadcast_to(w_n[:, None, :], (REL, HPC, N))).astype(f16)
    in_maps = []
    for c in range(N_CORES):
        b = c // (N_CORES // B)
        h0 = (c % (N_CORES // B)) * HPC
        cols = slice(D * h0, D * h0 + D * HPC)
        wkv = np.concatenate([Wk[:, cols], Wv[:, cols]], axis=1)
        bq6 = np.concatenate(
            [(bq[cols] * scale).reshape(HPC, D).T, bk[cols].reshape(HPC, D).T],
            axis=1)
        in_maps.append({
            "qT": qTs[b],
            "kvT": kvTs[b],
            "wqp": _to_p128(Wq[:, cols] * scale),
            "wkv": _to_p128(wkv),
            "wo": np.ascontiguousarray(
                Wo[cols, :].reshape(HPC, D, C).transpose(1, 0, 2)).astype(f16),
            "bq6": np.ascontiguousarray(bq6, dtype=np.float32),
            "bvb": np.ascontiguousarray(
                np.broadcast_to(bv[cols][None, :], (128, D * HPC))),
            "b1t": np.ascontiguousarray(
                b1[h0:h0 + HPC].transpose(2, 0, 1)).astype(f16),
            "w_n3": w_n3,
        })
    return in_maps


def kernel(query, key_value, Wq, bq, Wk, bk, Wv, bv, Wo, bo, rel_pos_bias):
    from concourse import bass_utils

    query = np.asarray(query, np.float32)
    key_value = np.asarray(key_value, np.float32)
    Wq = np.asarray(Wq, np.float32); bq = np.asarray(bq, np.float32)
    Wk = np.asarray(Wk, np.float32); bk = np.asarray(bk, np.float32)
    Wv = np.asarray(Wv, np.float32); bv = np.asarray(bv, np.float32)
    Wo = np.asarray(Wo, np.float32); bo = np.asarray(bo, np.float32)
    rel_pos_bias = np.asarray(rel_pos_bias, np.float32)

    in_maps = _make_in_maps(query, key_value, Wq, bq, Wk, bk, Wv, bv, Wo,
                            rel_pos_bias)
    nc = _get_compiled()
    res = bass_utils.run_bass_kernel_spmd(nc, in_maps,
                                          core_ids=list(range(N_CORES)))
    out = np.zeros((B, Q, C), np.float32)
    for c in range(N_CORES):
        b = c // (N_CORES // B)
        f = res.results[c]["out_p"].astype(np.float32)  # [HPC,Q,C] unnorm.
        d = res.results[c]["den"]            # [HPC, Q]
        out[b] += (f / d[:, :, None]).sum(axis=0)
    out += bo[None, None, :]
    return out


# revision 38
# speedup vs baseline: 1.0832x; 1.0832x over previous
"""Cross-attention (B=2, Q=1024, N=4096, C=768, H=12, D=64) with bilinearly
interpolated relative position bias, on 8 Trainium2 NeuronCores.

Sharding: core c handles batch b = c//4 and heads 3*(c%4) .. 3*(c%4)+2
(tensor-parallel over heads, data-parallel over batch). Each core outputs, per
head, the unnormalized attention output projected through Wo_h, plus the
softmax denominators; the host divides, sums the partials, and adds bo.

Device algorithm per core (fp16 matmul operands, fp32 accumulation):
  qbT[h]  = (Wq_h^T @ q^T) * scale + bq          [64, 1024]   (d-major)
  kbT[h]  = Wk_h^T @ kv^T + bk                   [64, 4096]
  vb[n]   = kv @ Wv_h + bv                       [4096, 64]   (n-major)
  S^T     = [kbT; Wn]^T-contraction [qbT; B1T]   K=96 fuses the interpolated
            bias: bias[h,q,n] = sum_j B1[h,q,j] * Wn[j,n]
  E^T     = exp(S^T)            (no max-subtraction; logits are O(1))
  O^T[h]  = [vb_h | 1]^T @ E^T                   [65, 1024]  row 64 = denom
  F[h]    = O^T[h]^T-contraction Wo_h            [1024, 768]  (unnormalized)

Schedule: pass1 = heads 0+1 S/exp interleaved with the streamed k/v
projections (O accumulated for h0 only; h1's E tiles stored in SBUF), pass2 =
h2 attention + h1's O from the store + Wo(h0,h1) interleaved, tail = Wo(h2).
This keeps the scalar engine (exp) saturated during the projection-heavy pass
and the PE saturated during the exp-heavy pass. DMA triggers cost ~0.7us each
on the SP sequencer, so inputs are packed into few large transfers and the
constant rows (interp weights, bias rows, denominator ones) are written with
one DMA / one ALU op instead of per-tile transfers.
"""

import numpy as np

B, Q, N, C = 2, 1024, 4096, 768
H, D, REL = 12, 64, 32
SCALE = 1.0 / np.sqrt(D)
HPC = 3            # heads per core
N_CORES = 8
NCH = 8            # 512-wide n-chunks
NCI = N // 128

_COMPILED = None   # cached nc across kernel() calls


def _lin_coords(n_out, n_in):
    pos = np.arange(n_out, dtype=np.float32) * np.float32((n_in - 1) / (n_out - 1))
    lo = np.clip(np.floor(pos).astype(np.int32), 0, n_in - 1)
    hi = np.clip(lo + 1, 0, n_in - 1)
    w = (pos - lo.astype(np.float32)).astype(np.float32)
    return lo, hi, w


def _host_bias_parts(rel_pos_bias):
    """B1: [H, Q, 32] q-interpolated bias; Wn: [32, N] n-interp weights."""
    lq, hq, wq = _lin_coords(Q, REL)
    ln, hn, wn = _lin_coords(N, REL)
    b1 = (rel_pos_bias[:, lq, :] * (1.0 - wq)[None, :, None]
          + rel_pos_bias[:, hq, :] * wq[None, :, None]).astype(np.float32)
    w_n = np.zeros((REL, N), np.float32)
    np.add.at(w_n, (ln, np.arange(N)), (1.0 - wn))
    np.add.at(w_n, (hn, np.arange(N)), wn)
    return b1, w_n


def _build():
    import concourse.tile as tile
    from concourse import bacc, mybir
    import concourse.bass as bass

    F32 = mybir.dt.float32
    F16 = mybir.dt.float16
    KT = 6  # C // 128 contraction tiles

    nc = bacc.Bacc("TRN2", target_bir_lowering=False, debug=False,
                   enable_asserts=False, num_devices=N_CORES)

    qT = nc.dram_tensor("qT", [128, 2, KT, 512], F16, kind="ExternalInput")
    kvT = nc.dram_tensor("kvT", [128, NCH, KT, 512], F16,
                         kind="ExternalInput")
    wqp = nc.dram_tensor("wqp", [128, KT, 192], F16, kind="ExternalInput")
    wkv = nc.dram_tensor("wkv", [128, KT, 384], F16, kind="ExternalInput")
    wo = nc.dram_tensor("wo", [D, HPC, C], F16, kind="ExternalInput")
    bq6 = nc.dram_tensor("bq6", [D, 2 * HPC], F32, kind="ExternalInput")
    bvb = nc.dram_tensor("bvb", [128, 192], F32, kind="ExternalInput")  # bcast bv
    b1t = nc.dram_tensor("b1t", [REL, HPC, Q], F16, kind="ExternalInput")
    w_n3 = nc.dram_tensor("w_n3", [REL, HPC, N], F16, kind="ExternalInput")
    out_p = nc.dram_tensor("out_p", [HPC, Q, C], F16, kind="ExternalOutput")
    den = nc.dram_tensor("den", [HPC, Q], F32, kind="ExternalOutput")

    EXP = mybir.ActivationFunctionType.Exp
    ADD = mybir.AluOpType.add
    MULT = mybir.AluOpType.mult

    with tile.TileContext(nc) as tc:
        with (
            tc.tile_pool(name="wpool", bufs=1) as wpool,
            tc.tile_pool(name="persist", bufs=1) as pers,
            tc.tile_pool(name="stream", bufs=2) as stream,
            tc.tile_pool(name="pexp", bufs=3) as pexp,
            tc.tile_pool(name="tailp", bufs=2) as tailp,
        ):
            # ---- weights: wq first (unblocks Q-proj), wo deferred ----
            wq_sb = wpool.tile([128, KT, 192], F16, name="wq_sb")
            nc.sync.dma_start(out=wq_sb, in_=wqp[:, :, :])
            bq6_sb = wpool.tile([D, 2 * HPC], F32, name="bq6_sb")
            nc.sync.dma_start(out=bq6_sb, in_=bq6[:, :])
            bqs_sb = bq6_sb[:, 0:HPC]
            bks_sb = bq6_sb[:, HPC:2 * HPC]

            # ---- persistent per-head tiles ----
            qTp = pers.tile([96, HPC, Q], F16, name="qTp", tag="qTp")
            kbT = pers.tile([96, HPC, NCH, 512], F16, name="kbT", tag="kbT")
            vb = pers.tile([128, NCI, HPC, 65], F16, name="vb", tag="vb")
            px1s = pers.tile([128, NCI, Q], F16, name="px1s", tag="px1s")

            # ---- phase A: q projection (qT loaded in contiguous halves) ----
            def load_kv(ch):
                kvc = stream.tile([128, KT, 512], F16, name="kvc", tag="kvc")
                nc.sync.dma_start(out=kvc, in_=kvT[:, ch])
                return kvc

            P = {}  # current transient-PSUM pool, swapped per pass

            with tc.tile_pool(name="qload", bufs=1) as qload, \
                 tc.tile_pool(name="psA", bufs=1, space="PSUM") as psA:
                qT_sb = qload.tile([128, 2, KT, 512], F16, name="qT_sb")
                nc.sync.dma_start(out=qT_sb[:, 0], in_=qT[:, 0])
                wkv_sb = wpool.tile([128, KT, 384], F16, name="wkv_sb")
                nc.sync.dma_start(out=wkv_sb, in_=wkv[:, :, :])
                wk_sb = wkv_sb[:, :, 0:192]
                wv_sb = wkv_sb[:, :, 192:384]
                # prefetch kv chunks 0/1 behind the Q-proj-critical loads
                kvc_pre = [load_kv(0), load_kv(1)]
                nc.sync.dma_start(out=qT_sb[:, 1], in_=qT[:, 1])
                bvb_sb = wpool.tile([128, 192], F32, name="bvb_sb")
                nc.sync.dma_start(out=bvb_sb, in_=bvb[:, :])
                nc.sync.dma_start(
                    out=kbT[64:96, :, :, :],
                    in_=w_n3.rearrange("p h (c n) -> p h c n", n=512))
                nc.sync.dma_start(out=qTp[64:96, :, :], in_=b1t[:, :, :])
                # denominator ones-rows of vb: single memset, no DMA
                nc.gpsimd.memset(vb[:, :, :, 64], 1.0)
                for qc in range(2):
                    sl = slice(512 * qc, 512 * qc + 512)
                    psqA = psA.tile([128, 512], F32, name="psqA", tag="psqA")
                    psqB = psA.tile([64, 512], F32, name="psqB", tag="psqB")
                    for t in range(KT):
                        nc.tensor.matmul(psqA, wq_sb[:, t, 0:128],
                                         qT_sb[:, qc, t, :],
                                         start=(t == 0), stop=(t == KT - 1))
                    for t in range(KT):
                        nc.tensor.matmul(psqB, wq_sb[:, t, 128:192],
                                         qT_sb[:, qc, t, :],
                                         start=(t == 0), stop=(t == KT - 1))
                    nc.vector.tensor_scalar_add(qTp[0:64, 0, sl], psqA[0:64, :],
                                                bqs_sb[:, 0:1])
                    nc.vector.tensor_scalar_add(qTp[0:64, 1, sl], psqA[64:128, :],
                                                bqs_sb[:, 1:2])
                    nc.vector.tensor_scalar_add(qTp[0:64, 2, sl], psqB[0:64, :],
                                                bqs_sb[:, 2:3])

            # ---- phase B chunk: k/v projections for one 512-wide n-chunk ----
            def emit_b_chunk(ch, kvc):
                psB = P['b']
                pskA = psB.tile([128, 512], F32, name="pskA", tag="psb")
                for t in range(KT):
                    nc.tensor.matmul(pskA, wk_sb[:, t, 0:128], kvc[:, t, :],
                                     start=(t == 0), stop=(t == KT - 1))
                nc.vector.tensor_scalar_add(kbT[0:64, 0, ch, :], pskA[0:64, :],
                                            bks_sb[:, 0:1])
                nc.vector.tensor_scalar_add(kbT[0:64, 1, ch, :], pskA[64:128, :],
                                            bks_sb[:, 1:2])
                pskB = psB.tile([64, 512], F32, name="pskB", tag="psb")
                for t in range(KT):
                    nc.tensor.matmul(pskB, wk_sb[:, t, 128:192], kvc[:, t, :],
                                     start=(t == 0), stop=(t == KT - 1))
                nc.vector.tensor_scalar_add(kbT[0:64, 2, ch, :], pskB[0:64, :],
                                            bks_sb[:, 2:3])
                for s in range(4):
                    n128 = 4 * ch + s
                    psv = psB.tile([128, 192], F32, name="psv", tag="psb")
                    for t in range(KT):
                        nc.tensor.matmul(psv, kvc[:, t, 128 * s:128 * s + 128],
                                         wv_sb[:, t, :],
                                         start=(t == 0), stop=(t == KT - 1))
                    nc.vector.tensor_tensor(
                        out=vb[:, n128, :, 0:64],
                        in0=psv.rearrange("p (h d) -> p h d", d=64),
                        in1=bvb_sb.rearrange("p (h d) -> p h d", d=64),
                        op=ADD)

            # ---- attention passes ----
            import contextlib

            def emit_s(h, ci):
                c512, s = ci // 4, ci % 4
                ssl = slice(128 * s, 128 * s + 128)
                psS = P['s'].tile([128, Q], F32, name="psS", tag="psS")
                nc.tensor.matmul(psS[:, 0:512], kbT[:, h, c512, ssl],
                                 qTp[:, h, 0:512], start=True, stop=True)
                nc.tensor.matmul(psS[:, 512:1024], kbT[:, h, c512, ssl],
                                 qTp[:, h, 512:1024], start=True, stop=True)
                return psS

            def emit_o(po, h, ci, src):
                st = (ci == 0)
                sp = (ci == NCI - 1)
                nc.tensor.matmul(po[:, 0:512], vb[:, ci, h, :],
                                 src[:, 0:512], start=st, stop=sp)
                nc.tensor.matmul(po[:, 512:1024], vb[:, ci, h, :],
                                 src[:, 512:1024], start=st, stop=sp)

            def emit_fin(po, h):
                onT = tailp.tile([64, Q], F16, name=f"onT{h}", tag=f"onT{h}",
                                 bufs=1)
                nc.vector.tensor_copy(out=onT, in_=po[0:64, :])
                dsb = tailp.tile([1, Q], F32, name=f"dsb{h}", tag="dsb", bufs=2)
                nc.scalar.copy(out=dsb, in_=po[64:65, :])
                nc.sync.dma_start(out=den[h, :], in_=dsb)
                return onT

            def wo_half(psF, h, onT, qt, osb, tail=False):
                tsl = slice(128 * qt, 128 * qt + 128)
                for nc2 in range(2):
                    nsl = slice(384 * nc2, 384 * nc2 + 384)
                    psf = psF.tile([128, 384], F32, name="psf", tag="psf")
                    nc.tensor.matmul(psf, onT[:, tsl], wo_sb[:, h, nsl],
                                     start=True, stop=True)
                    # in-pass copies stay on DVE (scalar is exp-bound there);
                    # tail copies alternate DVE/scalar (both engines idle)
                    if tail and nc2 == 1:
                        nc.scalar.copy(out=osb[:, qt % 2, nsl], in_=psf)
                    else:
                        nc.vector.tensor_copy(out=osb[:, qt % 2, nsl], in_=psf)
                if qt % 2 == 1:
                    nc.sync.dma_start(
                        out=out_p[h, 256 * (qt // 2):256 * (qt // 2) + 256, :]
                            .rearrange("(a p) c -> p a c", p=128),
                        in_=osb)

            # ---- pass 1: h0 + h1 (S/exp), O for h0; b-chunk projections ----
            with contextlib.ExitStack() as pstk:
                psO0 = pstk.enter_context(
                    tc.tile_pool(name="psO0", bufs=1, space="PSUM"))
                po0 = psO0.tile([65, Q], F32, name="po0", tag="po")
                P['s'] = pstk.enter_context(
                    tc.tile_pool(name="psP1", bufs=2, space="PSUM"))
                P['b'] = pstk.enter_context(
                    tc.tile_pool(name="psB", bufs=2, space="PSUM"))
                emit_b_chunk(0, kvc_pre[0])
                psS0_cur = emit_s(0, 0)
                psS1_cur = emit_s(1, 0)
                for ci in range(NCI):
                    if ci + 1 < NCI:
                        if (ci + 1) % 4 == 0:
                            ch = (ci + 1) // 4
                            if ch + 1 < NCH:
                                kvc_pre.append(load_kv(ch + 1))
                            emit_b_chunk(ch, kvc_pre[ch])
                        psS0_nxt = emit_s(0, ci + 1)
                        psS1_nxt = emit_s(1, ci + 1)
                    px0 = pexp.tile([128, Q], F16, name="px0", tag="px")
                    nc.scalar.activation(out=px0, in_=psS0_cur, func=EXP)
                    nc.scalar.activation(out=px1s[:, ci, :], in_=psS1_cur,
                                         func=EXP)
                    emit_o(po0, 0, ci, px0)
                    if ci + 1 < NCI:
                        psS0_cur = psS0_nxt
                        psS1_cur = psS1_nxt
                onT0 = emit_fin(po0, 0)

            # wo loaded between passes (SP queue idle here)
            wo_sb = wpool.tile([D, HPC, C], F16, name="wo_sb")
            nc.sync.dma_start(out=wo_sb, in_=wo[:, :, :])

            # ---- pass 2: h2 attention + h1's O + Wo(h0, h1) ----
            with contextlib.ExitStack() as pstk:
                psO2 = pstk.enter_context(
                    tc.tile_pool(name="psO2", bufs=1, space="PSUM"))
                po2 = psO2.tile([65, Q], F32, name="po2", tag="po")
                P['s'] = pstk.enter_context(
                    tc.tile_pool(name="psP2", bufs=2, space="PSUM"))
                psS2_cur = emit_s(2, 0)
                onT1 = None
                psF = None
                osb = None
                with contextlib.ExitStack() as pstk1:
                    psO1 = pstk1.enter_context(
                        tc.tile_pool(name="psO1", bufs=1, space="PSUM"))
                    po1 = psO1.tile([65, Q], F32, name="po1", tag="po")
                    for ci in range(NCI):
                        if ci + 1 < NCI:
                            psS2_nxt = emit_s(2, ci + 1)
                        px2 = pexp.tile([128, Q], F16, name="px2", tag="px")
                        nc.scalar.activation(out=px2, in_=psS2_cur, func=EXP)
                        emit_o(po2, 2, ci, px2)
                        if ci < 16:
                            emit_o(po1, 1, 2 * ci, px1s[:, 2 * ci, :])
                            emit_o(po1, 1, 2 * ci + 1, px1s[:, 2 * ci + 1, :])
                            if ci == 15:
                                onT1 = emit_fin(po1, 1)
                                pstk1.close()
                                psF = pstk.enter_context(
                                    tc.tile_pool(name="psF", bufs=2,
                                                 space="PSUM"))
                        else:
                            qt = ci - 16
                            hh, qtt = (0, qt) if qt < 8 else (1, qt - 8)
                            if qtt % 2 == 0:
                                osb = tailp.tile([128, 2, C], F16, name="osb",
                                                 tag="osb", bufs=2)
                            wo_half(psF, hh, onT0 if hh == 0 else onT1,
                                    qtt, osb)
                        if ci + 1 < NCI:
                            psS2_cur = psS2_nxt
                onT2 = emit_fin(po2, 2)
            with tc.tile_pool(name="psFt", bufs=3, space="PSUM") as psFt:
                for qt in range(8):
                    if qt % 2 == 0:
                        osb = tailp.tile([128, 2, C], F16, name="osb",
                                         tag="osb", bufs=2)
                    wo_half(psFt, 2, onT2, qt, osb, tail=True)
    nc.compile()
    return nc


def _get_compiled():
    global _COMPILED
    if _COMPILED is None:
        _COMPILED = _build()
    return _COMPILED


def _to_p128(a):
    """[768, M] -> [128, 6, M] partition-major fp16."""
    return np.ascontiguousarray(
        a.reshape(6, 128, -1).transpose(1, 0, 2)).astype(np.float16)


def _chunked(a, w):
    """[128, 6, M] -> [128, M//w, 6, w] (w-wide column chunks contiguous)."""
    p, t, m = a.shape
    return np.ascontiguousarray(
        a.reshape(p, t, m // w, w).transpose(0, 2, 1, 3))


def _make_in_maps(query, key_value, Wq, bq, Wk, bk, Wv, bv, Wo, rel_pos_bias):
    b1, w_n = _host_bias_parts(rel_pos_bias)
    scale = np.float32(SCALE)
    f16 = np.float16
    qTs = [_chunked(_to_p128(np.ascontiguousarray(query[b].T)), 512)
           for b in range(B)]
    kvTs = [_chunked(_to_p128(np.ascontiguousarray(key_value[b].T)), 512)
            for b in range(B)]
    w_n3 = np.ascontiguousarray(
        np.broadcast_to(w_n[:, None, :], (REL, HPC, N))).astype(f16)
    in_maps = []
    for c in range(N_CORES):
        b = c // (N_CORES // B)
        h0 = (c % (N_CORES // B)) * HPC
        cols = slice(D * h0, D * h0 + D * HPC)
        wkv = np.concatenate([Wk[:, cols], Wv[:, cols]], axis=1)
        bq6 = np.concatenate(
            [(bq[cols] * scale).reshape(HPC, D).T, bk[cols].reshape(HPC, D).T],
            axis=1)
        in_maps.append({
            "qT": qTs[b],
            "kvT": kvTs[b],
            "wqp": _to_p128(Wq[:, cols] * scale),
            "wkv": _to_p128(wkv),
            "wo": np.ascontiguousarray(
                Wo[cols, :].reshape(HPC, D, C).transpose(1, 0, 2)).astype(f16),
            "bq6": np.ascontiguousarray(bq6, dtype=np.float32),
            "bvb": np.ascontiguousarray(
                np.broadcast_to(bv[cols][None, :], (128, D * HPC))),
            "b1t": np.ascontiguousarray(
                b1[h0:h0 + HPC].transpose(2, 0, 1)).astype(f16),
            "w_n3": w_n3,
        })
    return in_maps


def kernel(query, key_value, Wq, bq, Wk, bk, Wv, bv, Wo, bo, rel_pos_bias):
    from concourse import bass_utils

    query = np.asarray(query, np.float32)
    key_value = np.asarray(key_value, np.float32)
    Wq = np.asarray(Wq, np.float32); bq = np.asarray(bq, np.float32)
    Wk = np.asarray(Wk, np.float32); bk = np.asarray(bk, np.float32)
    Wv = np.asarray(Wv, np.float32); bv = np.asarray(bv, np.float32)
    Wo = np.asarray(Wo, np.float32); bo = np.asarray(bo, np.float32)
    rel_pos_bias = np.asarray(rel_pos_bias, np.float32)

    in_maps = _make_in_maps(query, key_value, Wq, bq, Wk, bk, Wv, bv, Wo,
                            rel_pos_bias)
    nc = _get_compiled()
    res = bass_utils.run_bass_kernel_spmd(nc, in_maps,
                                          core_ids=list(range(N_CORES)))
    out = np.zeros((B, Q, C), np.float32)
    for c in range(N_CORES):
        b = c // (N_CORES // B)
        f = res.results[c]["out_p"].astype(np.float32)  # [HPC,Q,C] unnorm.
        d = res.results[c]["den"]            # [HPC, Q]
        out[b] += (f / d[:, :, None]).sum(axis=0)
    out += bo[None, None, :]
    return out


# revision 40
# speedup vs baseline: 1.0937x; 1.0097x over previous
"""Cross-attention (B=2, Q=1024, N=4096, C=768, H=12, D=64) with bilinearly
interpolated relative position bias, on 8 Trainium2 NeuronCores.

Sharding: core c handles batch b = c//4 and heads 3*(c%4) .. 3*(c%4)+2
(tensor-parallel over heads, data-parallel over batch). Each core outputs, per
head, the unnormalized attention output projected through Wo_h, plus the
softmax denominators; the host divides, sums the partials, and adds bo.

Device algorithm per core (fp16 matmul operands, fp32 accumulation):
  qbT[h]  = (Wq_h^T @ q^T) * scale + bq          [64, 1024]   (d-major)
  kbT[h]  = Wk_h^T @ kv^T + bk                   [64, 4096]
  vb[n]   = kv @ Wv_h + bv                       [4096, 64]   (n-major)
  S^T     = [kbT; Wn]^T-contraction [qbT; B1T]   K=96 fuses the interpolated
            bias: bias[h,q,n] = sum_j B1[h,q,j] * Wn[j,n]
  E^T     = exp(S^T)            (no max-subtraction; logits are O(1))
  O^T[h]  = [vb_h | 1]^T @ E^T                   [65, 1024]  row 64 = denom
  F[h]    = O^T[h]^T-contraction Wo_h            [1024, 768]  (unnormalized)

Schedule: pass1 = heads 0+1 S/exp interleaved with the streamed k/v
projections (O accumulated for h0 only; h1's E tiles stored in SBUF), pass2 =
h2 attention + h1's O from the store + Wo(h0,h1) interleaved, tail = Wo(h2).
This keeps the scalar engine (exp) saturated during the projection-heavy pass
and the PE saturated during the exp-heavy pass. DMA triggers cost ~0.7us each
on the SP sequencer, so inputs are packed into few large transfers and the
constant rows (interp weights, bias rows, denominator ones) are written with
one DMA / one ALU op instead of per-tile transfers.
"""

import numpy as np

B, Q, N, C = 2, 1024, 4096, 768
H, D, REL = 12, 64, 32
SCALE = 1.0 / np.sqrt(D)
HPC = 3            # heads per core
N_CORES = 8
NCH = 8            # 512-wide n-chunks
NCI = N // 128

_COMPILED = None   # cached nc across kernel() calls


def _lin_coords(n_out, n_in):
    pos = np.arange(n_out, dtype=np.float32) * np.float32((n_in - 1) / (n_out - 1))
    lo = np.clip(np.floor(pos).astype(np.int32), 0, n_in - 1)
    hi = np.clip(lo + 1, 0, n_in - 1)
    w = (pos - lo.astype(np.float32)).astype(np.float32)
    return lo, hi, w


def _host_bias_parts(rel_pos_bias):
    """B1: [H, Q, 32] q-interpolated bias; Wn: [32, N] n-interp weights."""
    lq, hq, wq = _lin_coords(Q, REL)
    ln, hn, wn = _lin_coords(N, REL)
    b1 = (rel_pos_bias[:, lq, :] * (1.0 - wq)[None, :, None]
          + rel_pos_bias[:, hq, :] * wq[None, :, None]).astype(np.float32)
    w_n = np.zeros((REL, N), np.float32)
    np.add.at(w_n, (ln, np.arange(N)), (1.0 - wn))
    np.add.at(w_n, (hn, np.arange(N)), wn)
    return b1, w_n


def _build():
    import concourse.tile as tile
    from concourse import bacc, mybir
    import concourse.bass as bass

    F32 = mybir.dt.float32
    F16 = mybir.dt.float16
    KT = 6  # C // 128 contraction tiles

    nc = bacc.Bacc("TRN2", target_bir_lowering=False, debug=False,
                   enable_asserts=False, num_devices=N_CORES)

    qT = nc.dram_tensor("qT", [128, 2, KT, 512], F16, kind="ExternalInput")
    kvT = nc.dram_tensor("kvT", [128, NCH, KT, 512], F16,
                         kind="ExternalInput")
    wqp = nc.dram_tensor("wqp", [128, KT, 192], F16, kind="ExternalInput")
    wkv = nc.dram_tensor("wkv", [128, KT, 384], F16, kind="ExternalInput")
    wo = nc.dram_tensor("wo", [D, HPC, C], F16, kind="ExternalInput")
    bq6 = nc.dram_tensor("bq6", [D, 2 * HPC], F32, kind="ExternalInput")
    bvb = nc.dram_tensor("bvb", [128, 192], F32, kind="ExternalInput")  # bcast bv
    b1t = nc.dram_tensor("b1t", [REL, HPC, Q], F16, kind="ExternalInput")
    w_n3 = nc.dram_tensor("w_n3", [REL, HPC, N], F16, kind="ExternalInput")
    out_p = nc.dram_tensor("out_p", [HPC, Q, C], F16, kind="ExternalOutput")
    den = nc.dram_tensor("den", [HPC, Q], F32, kind="ExternalOutput")

    EXP = mybir.ActivationFunctionType.Exp
    ADD = mybir.AluOpType.add
    MULT = mybir.AluOpType.mult

    with tile.TileContext(nc) as tc:
        with (
            tc.tile_pool(name="wpool", bufs=1) as wpool,
            tc.tile_pool(name="persist", bufs=1) as pers,
            tc.tile_pool(name="stream", bufs=2) as stream,
            tc.tile_pool(name="pexp", bufs=3) as pexp,
            tc.tile_pool(name="tailp", bufs=2) as tailp,
        ):
            # ---- weights: wq first (unblocks Q-proj), wo deferred ----
            wq_sb = wpool.tile([128, KT, 192], F16, name="wq_sb")
            nc.sync.dma_start(out=wq_sb, in_=wqp[:, :, :])
            bq6_sb = wpool.tile([D, 2 * HPC], F32, name="bq6_sb")
            nc.sync.dma_start(out=bq6_sb, in_=bq6[:, :])
            bqs_sb = bq6_sb[:, 0:HPC]
            bks_sb = bq6_sb[:, HPC:2 * HPC]

            # ---- persistent per-head tiles ----
            qTp = pers.tile([96, HPC, Q], F16, name="qTp", tag="qTp")
            kbT = pers.tile([96, HPC, NCH, 512], F16, name="kbT", tag="kbT")
            vb = pers.tile([128, NCI, HPC, 65], F16, name="vb", tag="vb")
            px1s = pers.tile([128, NCI, Q], F16, name="px1s", tag="px1s")

            # ---- phase A: q projection (qT loaded in contiguous halves) ----
            def load_kv(ch):
                kvc = stream.tile([128, KT, 512], F16, name="kvc", tag="kvc")
                nc.sync.dma_start(out=kvc, in_=kvT[:, ch])
                return kvc

            P = {}  # current transient-PSUM pool, swapped per pass

            with tc.tile_pool(name="qload", bufs=1) as qload, \
                 tc.tile_pool(name="psA", bufs=1, space="PSUM") as psA:
                qT_sb = qload.tile([128, 2, KT, 512], F16, name="qT_sb")
                nc.sync.dma_start(out=qT_sb[:, 0], in_=qT[:, 0])
                wkv_sb = wpool.tile([128, KT, 384], F16, name="wkv_sb")
                nc.sync.dma_start(out=wkv_sb, in_=wkv[:, :, :])
                wk_sb = wkv_sb[:, :, 0:192]
                wv_sb = wkv_sb[:, :, 192:384]
                # prefetch kv chunks 0/1 behind the Q-proj-critical loads
                kvc_pre = [load_kv(0), load_kv(1)]
                nc.sync.dma_start(out=qT_sb[:, 1], in_=qT[:, 1])
                bvb_sb = wpool.tile([128, 192], F32, name="bvb_sb")
                nc.sync.dma_start(out=bvb_sb, in_=bvb[:, :])
                nc.sync.dma_start(
                    out=kbT[64:96, :, :, :],
                    in_=w_n3.rearrange("p h (c n) -> p h c n", n=512))
                nc.sync.dma_start(out=qTp[64:96, :, :], in_=b1t[:, :, :])
                # denominator ones-rows of vb: single memset, no DMA
                nc.gpsimd.memset(vb[:, :, :, 64], 1.0)
                for qc in range(2):
                    sl = slice(512 * qc, 512 * qc + 512)
                    psqA = psA.tile([128, 512], F32, name="psqA", tag="psqA")
                    psqB = psA.tile([64, 512], F32, name="psqB", tag="psqB")
                    for t in range(KT):
                        nc.tensor.matmul(psqA, wq_sb[:, t, 0:128],
                                         qT_sb[:, qc, t, :],
                                         start=(t == 0), stop=(t == KT - 1))
                    for t in range(KT):
                        nc.tensor.matmul(psqB, wq_sb[:, t, 128:192],
                                         qT_sb[:, qc, t, :],
                                         start=(t == 0), stop=(t == KT - 1))
                    nc.vector.tensor_scalar_add(qTp[0:64, 0, sl], psqA[0:64, :],
                                                bqs_sb[:, 0:1])
                    nc.vector.tensor_scalar_add(qTp[0:64, 1, sl], psqA[64:128, :],
                                                bqs_sb[:, 1:2])
                    nc.vector.tensor_scalar_add(qTp[0:64, 2, sl], psqB[0:64, :],
                                                bqs_sb[:, 2:3])

            # ---- phase B chunk: k/v projections for one 512-wide n-chunk ----
            def emit_k_chunk(ch, kvc):
                psB = P['b']
                pskA = psB.tile([128, 512], F32, name="pskA", tag="psb")
                for t in range(KT):
                    nc.tensor.matmul(pskA, wk_sb[:, t, 0:128], kvc[:, t, :],
                                     start=(t == 0), stop=(t == KT - 1))
                nc.vector.tensor_scalar_add(kbT[0:64, 0, ch, :], pskA[0:64, :],
                                            bks_sb[:, 0:1])
                nc.vector.tensor_scalar_add(kbT[0:64, 1, ch, :], pskA[64:128, :],
                                            bks_sb[:, 1:2])
                pskB = psB.tile([64, 512], F32, name="pskB", tag="psb")
                for t in range(KT):
                    nc.tensor.matmul(pskB, wk_sb[:, t, 128:192], kvc[:, t, :],
                                     start=(t == 0), stop=(t == KT - 1))
                nc.vector.tensor_scalar_add(kbT[0:64, 2, ch, :], pskB[0:64, :],
                                            bks_sb[:, 2:3])

            def emit_v_chunk(ch, kvc):
                psB = P['b']
                for s in range(4):
                    n128 = 4 * ch + s
                    psv = psB.tile([128, 192], F32, name="psv", tag="psb")
                    for t in range(KT):
                        nc.tensor.matmul(psv, kvc[:, t, 128 * s:128 * s + 128],
                                         wv_sb[:, t, :],
                                         start=(t == 0), stop=(t == KT - 1))
                    nc.vector.tensor_tensor(
                        out=vb[:, n128, :, 0:64],
                        in0=psv.rearrange("p (h d) -> p h d", d=64),
                        in1=bvb_sb.rearrange("p (h d) -> p h d", d=64),
                        op=ADD)

            # ---- attention passes ----
            import contextlib

            def emit_s(h, ci):
                c512, s = ci // 4, ci % 4
                ssl = slice(128 * s, 128 * s + 128)
                psS = P['s'].tile([128, Q], F32, name="psS", tag="psS")
                nc.tensor.matmul(psS[:, 0:512], kbT[:, h, c512, ssl],
                                 qTp[:, h, 0:512], start=True, stop=True)
                nc.tensor.matmul(psS[:, 512:1024], kbT[:, h, c512, ssl],
                                 qTp[:, h, 512:1024], start=True, stop=True)
                return psS

            def emit_o(po, h, ci, src):
                st = (ci == 0)
                sp = (ci == NCI - 1)
                nc.tensor.matmul(po[:, 0:512], vb[:, ci, h, :],
                                 src[:, 0:512], start=st, stop=sp)
                nc.tensor.matmul(po[:, 512:1024], vb[:, ci, h, :],
                                 src[:, 512:1024], start=st, stop=sp)

            def emit_fin(po, h):
                onT = tailp.tile([64, Q], F16, name=f"onT{h}", tag=f"onT{h}",
                                 bufs=1)
                nc.vector.tensor_copy(out=onT, in_=po[0:64, :])
                dsb = tailp.tile([1, Q], F32, name=f"dsb{h}", tag="dsb", bufs=2)
                nc.scalar.copy(out=dsb, in_=po[64:65, :])
                nc.sync.dma_start(out=den[h, :], in_=dsb)
                return onT

            def wo_half(psF, h, onT, qt, osb, tail=False):
                tsl = slice(128 * qt, 128 * qt + 128)
                for nc2 in range(2):
                    nsl = slice(384 * nc2, 384 * nc2 + 384)
                    psf = psF.tile([128, 384], F32, name="psf", tag="psf")
                    nc.tensor.matmul(psf, onT[:, tsl], wo_sb[:, h, nsl],
                                     start=True, stop=True)
                    # in-pass copies stay on DVE (scalar is exp-bound there);
                    # tail copies alternate DVE/scalar (both engines idle)
                    if tail and nc2 == 1:
                        nc.scalar.copy(out=osb[:, qt % 2, nsl], in_=psf)
                    else:
                        nc.vector.tensor_copy(out=osb[:, qt % 2, nsl], in_=psf)
                if qt % 2 == 1:
                    nc.sync.dma_start(
                        out=out_p[h, 256 * (qt // 2):256 * (qt // 2) + 256, :]
                            .rearrange("(a p) c -> p a c", p=128),
                        in_=osb)

            # ---- pass 1: h0 + h1 (S/exp), O for h0; b-chunk projections ----
            with contextlib.ExitStack() as pstk:
                psO0 = pstk.enter_context(
                    tc.tile_pool(name="psO0", bufs=1, space="PSUM"))
                po0 = psO0.tile([65, Q], F32, name="po0", tag="po")
                P['s'] = pstk.enter_context(
                    tc.tile_pool(name="psP1", bufs=2, space="PSUM"))
                P['b'] = pstk.enter_context(
                    tc.tile_pool(name="psB", bufs=2, space="PSUM"))
                emit_k_chunk(0, kvc_pre[0])
                psS0_cur = emit_s(0, 0)
                psS1_cur = emit_s(1, 0)
                emit_v_chunk(0, kvc_pre[0])
                for ci in range(NCI):
                    if ci + 1 < NCI:
                        if (ci + 1) % 4 == 0:
                            ch = (ci + 1) // 4
                            if ch + 1 < NCH:
                                kvc_pre.append(load_kv(ch + 1))
                            emit_k_chunk(ch, kvc_pre[ch])
                        psS0_nxt = emit_s(0, ci + 1)
                        psS1_nxt = emit_s(1, ci + 1)
                        if (ci + 1) % 4 == 0:
                            emit_v_chunk((ci + 1) // 4, kvc_pre[(ci + 1) // 4])
                    px0 = pexp.tile([128, Q], F16, name="px0", tag="px")
                    nc.scalar.activation(out=px0, in_=psS0_cur, func=EXP)
                    nc.scalar.activation(out=px1s[:, ci, :], in_=psS1_cur,
                                         func=EXP)
                    emit_o(po0, 0, ci, px0)
                    if ci + 1 < NCI:
                        psS0_cur = psS0_nxt
                        psS1_cur = psS1_nxt
                onT0 = emit_fin(po0, 0)

            # wo loaded between passes (SP queue idle here)
            wo_sb = wpool.tile([D, HPC, C], F16, name="wo_sb")
            nc.sync.dma_start(out=wo_sb, in_=wo[:, :, :])

            # ---- pass 2: h2 attention + h1's O + Wo(h0, h1) ----
            with contextlib.ExitStack() as pstk:
                psO2 = pstk.enter_context(
                    tc.tile_pool(name="psO2", bufs=1, space="PSUM"))
                po2 = psO2.tile([65, Q], F32, name="po2", tag="po")
                P['s'] = pstk.enter_context(
                    tc.tile_pool(name="psP2", bufs=2, space="PSUM"))
                psS2_cur = emit_s(2, 0)
                onT1 = None
                psF = None
                osb = None
                with contextlib.ExitStack() as pstk1:
                    psO1 = pstk1.enter_context(
                        tc.tile_pool(name="psO1", bufs=1, space="PSUM"))
                    po1 = psO1.tile([65, Q], F32, name="po1", tag="po")
                    for ci in range(NCI):
                        if ci + 1 < NCI:
                            psS2_nxt = emit_s(2, ci + 1)
                        px2 = pexp.tile([128, Q], F16, name="px2", tag="px")
                        nc.scalar.activation(out=px2, in_=psS2_cur, func=EXP)
                        emit_o(po2, 2, ci, px2)
                        if ci < 16:
                            emit_o(po1, 1, 2 * ci, px1s[:, 2 * ci, :])
                            emit_o(po1, 1, 2 * ci + 1, px1s[:, 2 * ci + 1, :])
                            if ci == 15:
                                onT1 = emit_fin(po1, 1)
                                pstk1.close()
                                psF = pstk.enter_context(
                                    tc.tile_pool(name="psF", bufs=2,
                                                 space="PSUM"))
                        else:
                            qt = ci - 16
                            hh, qtt = (0, qt) if qt < 8 else (1, qt - 8)
                            if qtt % 2 == 0:
                                osb = tailp.tile([128, 2, C], F16, name="osb",
                                                 tag="osb", bufs=2)
                            wo_half(psF, hh, onT0 if hh == 0 else onT1,
                                    qtt, osb)
                        if ci + 1 < NCI:
                            psS2_cur = psS2_nxt
                onT2 = emit_fin(po2, 2)
            with tc.tile_pool(name="psFt", bufs=3, space="PSUM") as psFt:
                for qt in range(8):
                    if qt % 2 == 0:
                        osb = tailp.tile([128, 2, C], F16, name="osb",
                                         tag="osb", bufs=2)
                    wo_half(psFt, 2, onT2, qt, osb, tail=True)
    nc.compile()
    return nc


def _get_compiled():
    global _COMPILED
    if _COMPILED is None:
        _COMPILED = _build()
    return _COMPILED


def _to_p128(a):
    """[768, M] -> [128, 6, M] partition-major fp16."""
    return np.ascontiguousarray(
        a.reshape(6, 128, -1).transpose(1, 0, 2)).astype(np.float16)


def _chunked(a, w):
    """[128, 6, M] -> [128, M//w, 6, w] (w-wide column chunks contiguous)."""
    p, t, m = a.shape
    return np.ascontiguousarray(
        a.reshape(p, t, m // w, w).transpose(0, 2, 1, 3))


def _make_in_maps(query, key_value, Wq, bq, Wk, bk, Wv, bv, Wo, rel_pos_bias):
    b1, w_n = _host_bias_parts(rel_pos_bias)
    scale = np.float32(SCALE)
    f16 = np.float16
    qTs = [_chunked(_to_p128(np.ascontiguousarray(query[b].T)), 512)
           for b in range(B)]
    kvTs = [_chunked(_to_p128(np.ascontiguousarray(key_value[b].T)), 512)
            for b in range(B)]
    w_n3 = np.ascontiguousarray(
        np.broadcast_to(w_n[:, None, :], (REL, HPC, N))).astype(f16)
    in_maps = []
    for c in range(N_CORES):
        b = c // (N_CORES // B)
        h0 = (c % (N_CORES // B)) * HPC
        cols = slice(D * h0, D * h0 + D * HPC)
        wkv = np.concatenate([Wk[:, cols], Wv[:, cols]], axis=1)
        bq6 = np.concatenate(
            [(bq[cols] * scale).reshape(HPC, D).T, bk[cols].reshape(HPC, D).T],
            axis=1)
        in_maps.append({
            "qT": qTs[b],
            "kvT": kvTs[b],
            "wqp": _to_p128(Wq[:, cols] * scale),
            "wkv": _to_p128(wkv),
            "wo": np.ascontiguousarray(
                Wo[cols, :].reshape(HPC, D, C).transpose(1, 0, 2)).astype(f16),
            "bq6": np.ascontiguousarray(bq6, dtype=np.float32),
            "bvb": np.ascontiguousarray(
                np.broadcast_to(bv[cols][None, :], (128, D * HPC))),
            "b1t": np.ascontiguousarray(
                b1[h0:h0 + HPC].transpose(2, 0, 1)).astype(f16),
            "w_n3": w_n3,
        })
    return in_maps


def kernel(query, key_value, Wq, bq, Wk, bk, Wv, bv, Wo, bo, rel_pos_bias):
    from concourse import bass_utils

    query = np.asarray(query, np.float32)
    key_value = np.asarray(key_value, np.float32)
    Wq = np.asarray(Wq, np.float32); bq = np.asarray(bq, np.float32)
    Wk = np.asarray(Wk, np.float32); bk = np.asarray(bk, np.float32)
    Wv = np.asarray(Wv, np.float32); bv = np.asarray(bv, np.float32)
    Wo = np.asarray(Wo, np.float32); bo = np.asarray(bo, np.float32)
    rel_pos_bias = np.asarray(rel_pos_bias, np.float32)

    in_maps = _make_in_maps(query, key_value, Wq, bq, Wk, bk, Wv, bv, Wo,
                            rel_pos_bias)
    nc = _get_compiled()
    res = bass_utils.run_bass_kernel_spmd(nc, in_maps,
                                          core_ids=list(range(N_CORES)))
    out = np.zeros((B, Q, C), np.float32)
    for c in range(N_CORES):
        b = c // (N_CORES // B)
        f = res.results[c]["out_p"].astype(np.float32)  # [HPC,Q,C] unnorm.
        d = res.results[c]["den"]            # [HPC, Q]
        out[b] += (f / d[:, :, None]).sum(axis=0)
    out += bo[None, None, :]
    return out


# revision 41
# speedup vs baseline: 1.1057x; 1.0110x over previous
"""Cross-attention (B=2, Q=1024, N=4096, C=768, H=12, D=64) with bilinearly
interpolated relative position bias, on 8 Trainium2 NeuronCores.

Sharding: core c handles batch b = c//4 and heads 3*(c%4) .. 3*(c%4)+2
(tensor-parallel over heads, data-parallel over batch). Each core outputs, per
head, the unnormalized attention output projected through Wo_h, plus the
softmax denominators; the host divides, sums the partials, and adds bo.

Device algorithm per core (fp16 matmul operands, fp32 accumulation):
  qbT[h]  = (Wq_h^T @ q^T) * scale + bq          [64, 1024]   (d-major)
  kbT[h]  = Wk_h^T @ kv^T + bk                   [64, 4096]
  vb[n]   = kv @ Wv_h + bv                       [4096, 64]   (n-major)
  S^T     = [kbT; Wn]^T-contraction [qbT; B1T]   K=96 fuses the interpolated
            bias: bias[h,q,n] = sum_j B1[h,q,j] * Wn[j,n]
  E^T     = exp(S^T)            (no max-subtraction; logits are O(1))
  O^T[h]  = [vb_h | 1]^T @ E^T                   [65, 1024]  row 64 = denom
  F[h]    = O^T[h]^T-contraction Wo_h            [1024, 768]  (unnormalized)

Schedule: pass1 = heads 0+1 S/exp interleaved with the streamed k/v
projections (O accumulated for h0 only; h1's E tiles stored in SBUF), pass2 =
h2 attention + h1's O from the store + Wo(h0,h1) interleaved, tail = Wo(h2).
This keeps the scalar engine (exp) saturated during the projection-heavy pass
and the PE saturated during the exp-heavy pass. DMA triggers cost ~0.7us each
on the SP sequencer, so inputs are packed into few large transfers and the
constant rows (interp weights, bias rows, denominator ones) are written with
one DMA / one ALU op instead of per-tile transfers.
"""

import numpy as np

B, Q, N, C = 2, 1024, 4096, 768
H, D, REL = 12, 64, 32
SCALE = 1.0 / np.sqrt(D)
HPC = 3            # heads per core
N_CORES = 8
NCH = 8            # 512-wide n-chunks
NCI = N // 128

_COMPILED = None   # cached nc across kernel() calls


def _lin_coords(n_out, n_in):
    pos = np.arange(n_out, dtype=np.float32) * np.float32((n_in - 1) / (n_out - 1))
    lo = np.clip(np.floor(pos).astype(np.int32), 0, n_in - 1)
    hi = np.clip(lo + 1, 0, n_in - 1)
    w = (pos - lo.astype(np.float32)).astype(np.float32)
    return lo, hi, w


def _host_bias_parts(rel_pos_bias):
    """B1: [H, Q, 32] q-interpolated bias; Wn: [32, N] n-interp weights."""
    lq, hq, wq = _lin_coords(Q, REL)
    ln, hn, wn = _lin_coords(N, REL)
    b1 = (rel_pos_bias[:, lq, :] * (1.0 - wq)[None, :, None]
          + rel_pos_bias[:, hq, :] * wq[None, :, None]).astype(np.float32)
    w_n = np.zeros((REL, N), np.float32)
    np.add.at(w_n, (ln, np.arange(N)), (1.0 - wn))
    np.add.at(w_n, (hn, np.arange(N)), wn)
    return b1, w_n


def _build():
    import concourse.tile as tile
    from concourse import bacc, mybir
    import concourse.bass as bass

    F32 = mybir.dt.float32
    F16 = mybir.dt.float16
    KT = 6  # C // 128 contraction tiles

    nc = bacc.Bacc("TRN2", target_bir_lowering=False, debug=False,
                   enable_asserts=False, num_devices=N_CORES)

    qT = nc.dram_tensor("qT", [128, 2, KT, 512], F16, kind="ExternalInput")
    kvT = nc.dram_tensor("kvT", [128, NCH, KT, 512], F16,
                         kind="ExternalInput")
    wqp = nc.dram_tensor("wqp", [128, KT, 192], F16, kind="ExternalInput")
    wkv = nc.dram_tensor("wkv", [128, KT, 384], F16, kind="ExternalInput")
    wo = nc.dram_tensor("wo", [D, HPC, C], F16, kind="ExternalInput")
    bq6 = nc.dram_tensor("bq6", [D, 2 * HPC], F32, kind="ExternalInput")
    bvb = nc.dram_tensor("bvb", [128, 192], F32, kind="ExternalInput")  # bcast bv
    b1t = nc.dram_tensor("b1t", [REL, HPC, Q], F16, kind="ExternalInput")
    w_n3 = nc.dram_tensor("w_n3", [REL, HPC, N], F16, kind="ExternalInput")
    out_p = nc.dram_tensor("out_p", [HPC, Q, C], F16, kind="ExternalOutput")
    den = nc.dram_tensor("den", [HPC, Q], F32, kind="ExternalOutput")

    EXP = mybir.ActivationFunctionType.Exp
    ADD = mybir.AluOpType.add
    MULT = mybir.AluOpType.mult

    with tile.TileContext(nc) as tc:
        with (
            tc.tile_pool(name="wpool", bufs=1) as wpool,
            tc.tile_pool(name="persist", bufs=1) as pers,
            tc.tile_pool(name="stream", bufs=2) as stream,
            tc.tile_pool(name="pexp", bufs=3) as pexp,
            tc.tile_pool(name="tailp", bufs=2) as tailp,
        ):
            # ---- weights: wq first (unblocks Q-proj), wo deferred ----
            wq_sb = wpool.tile([128, KT, 192], F16, name="wq_sb")
            nc.sync.dma_start(out=wq_sb, in_=wqp[:, :, :])
            bq6_sb = wpool.tile([D, 2 * HPC], F32, name="bq6_sb")
            nc.sync.dma_start(out=bq6_sb, in_=bq6[:, :])
            bqs_sb = bq6_sb[:, 0:HPC]
            bks_sb = bq6_sb[:, HPC:2 * HPC]

            # ---- persistent per-head tiles ----
            qTp = pers.tile([96, HPC, Q], F16, name="qTp", tag="qTp")
            kbT = pers.tile([96, HPC, NCH, 512], F16, name="kbT", tag="kbT")
            vb = pers.tile([128, NCI, HPC, 65], F16, name="vb", tag="vb")
            px1s = pers.tile([128, NCI, Q], F16, name="px1s", tag="px1s")

            # ---- phase A: q projection (qT loaded in contiguous halves) ----
            def load_kv(ch):
                kvc = stream.tile([128, KT, 512], F16, name="kvc", tag="kvc")
                nc.sync.dma_start(out=kvc, in_=kvT[:, ch])
                return kvc

            P = {}  # current transient-PSUM pool, swapped per pass

            with tc.tile_pool(name="qload", bufs=1) as qload, \
                 tc.tile_pool(name="psA", bufs=1, space="PSUM") as psA:
                qT_sb = qload.tile([128, 2, KT, 512], F16, name="qT_sb")
                nc.sync.dma_start(out=qT_sb[:, 0], in_=qT[:, 0])
                wkv_sb = wpool.tile([128, KT, 384], F16, name="wkv_sb")
                nc.sync.dma_start(out=wkv_sb, in_=wkv[:, :, :])
                wk_sb = wkv_sb[:, :, 0:192]
                wv_sb = wkv_sb[:, :, 192:384]
                # prefetch kv chunks 0/1 behind the Q-proj-critical loads
                kvc_pre = [load_kv(0), load_kv(1)]
                nc.sync.dma_start(out=qT_sb[:, 1], in_=qT[:, 1])
                bvb_sb = wpool.tile([128, 192], F32, name="bvb_sb")
                nc.sync.dma_start(out=bvb_sb, in_=bvb[:, :])
                nc.sync.dma_start(
                    out=kbT[64:96, :, :, :],
                    in_=w_n3.rearrange("p h (c n) -> p h c n", n=512))
                nc.sync.dma_start(out=qTp[64:96, :, :], in_=b1t[:, :, :])
                # denominator ones-rows of vb: single memset, no DMA
                nc.gpsimd.memset(vb[:, :, :, 64], 1.0)
                for qc in range(2):
                    sl = slice(512 * qc, 512 * qc + 512)
                    psqA = psA.tile([128, 512], F32, name="psqA", tag="psqA")
                    psqB = psA.tile([64, 512], F32, name="psqB", tag="psqB")
                    for t in range(KT):
                        nc.tensor.matmul(psqA, wq_sb[:, t, 0:128],
                                         qT_sb[:, qc, t, :],
                                         start=(t == 0), stop=(t == KT - 1))
                    for t in range(KT):
                        nc.tensor.matmul(psqB, wq_sb[:, t, 128:192],
                                         qT_sb[:, qc, t, :],
                                         start=(t == 0), stop=(t == KT - 1))
                    nc.vector.tensor_scalar_add(qTp[0:64, 0, sl], psqA[0:64, :],
                                                bqs_sb[:, 0:1])
                    nc.vector.tensor_scalar_add(qTp[0:64, 1, sl], psqA[64:128, :],
                                                bqs_sb[:, 1:2])
                    nc.vector.tensor_scalar_add(qTp[0:64, 2, sl], psqB[0:64, :],
                                                bqs_sb[:, 2:3])

            # ---- phase B chunk: k/v projections for one 512-wide n-chunk ----
            def emit_k_chunk(ch, kvc):
                psB = P['b']
                pskA = psB.tile([128, 512], F32, name="pskA", tag="psb")
                for t in range(KT):
                    nc.tensor.matmul(pskA, wk_sb[:, t, 0:128], kvc[:, t, :],
                                     start=(t == 0), stop=(t == KT - 1))
                nc.vector.tensor_scalar_add(kbT[0:64, 0, ch, :], pskA[0:64, :],
                                            bks_sb[:, 0:1])
                nc.vector.tensor_scalar_add(kbT[0:64, 1, ch, :], pskA[64:128, :],
                                            bks_sb[:, 1:2])
                pskB = psB.tile([64, 512], F32, name="pskB", tag="psb")
                for t in range(KT):
                    nc.tensor.matmul(pskB, wk_sb[:, t, 128:192], kvc[:, t, :],
                                     start=(t == 0), stop=(t == KT - 1))
                nc.vector.tensor_scalar_add(kbT[0:64, 2, ch, :], pskB[0:64, :],
                                            bks_sb[:, 2:3])

            def emit_v_chunk(ch, kvc):
                psB = P['b']
                for s in range(4):
                    n128 = 4 * ch + s
                    psv = psB.tile([128, 192], F32, name="psv", tag="psb")
                    for t in range(KT):
                        nc.tensor.matmul(psv, kvc[:, t, 128 * s:128 * s + 128],
                                         wv_sb[:, t, :],
                                         start=(t == 0), stop=(t == KT - 1))
                    nc.vector.tensor_tensor(
                        out=vb[:, n128, :, 0:64],
                        in0=psv.rearrange("p (h d) -> p h d", d=64),
                        in1=bvb_sb.rearrange("p (h d) -> p h d", d=64),
                        op=ADD)

            # ---- attention passes ----
            import contextlib

            def emit_s(h, ci):
                c512, s = ci // 4, ci % 4
                ssl = slice(128 * s, 128 * s + 128)
                psS = P['s'].tile([128, Q], F32, name="psS", tag="psS")
                nc.tensor.matmul(psS[:, 0:512], kbT[:, h, c512, ssl],
                                 qTp[:, h, 0:512], start=True, stop=True)
                nc.tensor.matmul(psS[:, 512:1024], kbT[:, h, c512, ssl],
                                 qTp[:, h, 512:1024], start=True, stop=True)
                return psS

            def emit_o(po, h, ci, src):
                st = (ci == 0)
                sp = (ci == NCI - 1)
                nc.tensor.matmul(po[:, 0:512], vb[:, ci, h, :],
                                 src[:, 0:512], start=st, stop=sp)
                nc.tensor.matmul(po[:, 512:1024], vb[:, ci, h, :],
                                 src[:, 512:1024], start=st, stop=sp)

            def emit_fin(po, h):
                onT = tailp.tile([64, Q], F16, name=f"onT{h}", tag=f"onT{h}",
                                 bufs=1)
                nc.vector.tensor_copy(out=onT, in_=po[0:64, :])
                dsb = tailp.tile([1, Q], F32, name=f"dsb{h}", tag="dsb", bufs=2)
                nc.scalar.copy(out=dsb, in_=po[64:65, :])
                nc.sync.dma_start(out=den[h, :], in_=dsb)
                return onT

            def wo_half(psF, h, onT, qt, osb, tail=False):
                tsl = slice(128 * qt, 128 * qt + 128)
                for nc2 in range(2):
                    nsl = slice(384 * nc2, 384 * nc2 + 384)
                    psf = psF.tile([128, 384], F32, name="psf", tag="psf")
                    nc.tensor.matmul(psf, onT[:, tsl], wo_sb[:, h, nsl],
                                     start=True, stop=True)
                    # in-pass copies stay on DVE (scalar is exp-bound there);
                    # tail copies alternate DVE/scalar (both engines idle)
                    if tail and nc2 == 1:
                        nc.scalar.copy(out=osb[:, qt % 2, nsl], in_=psf)
                    else:
                        nc.vector.tensor_copy(out=osb[:, qt % 2, nsl], in_=psf)
                if qt % 2 == 1:
                    nc.sync.dma_start(
                        out=out_p[h, 256 * (qt // 2):256 * (qt // 2) + 256, :]
                            .rearrange("(a p) c -> p a c", p=128),
                        in_=osb)

            # ---- pass 1: h0 + h1 (S/exp), O for h0; b-chunk projections ----
            with contextlib.ExitStack() as pstk:
                psO0 = pstk.enter_context(
                    tc.tile_pool(name="psO0", bufs=1, space="PSUM"))
                po0 = psO0.tile([65, Q], F32, name="po0", tag="po")
                P['s'] = pstk.enter_context(
                    tc.tile_pool(name="psP1", bufs=2, space="PSUM"))
                P['b'] = pstk.enter_context(
                    tc.tile_pool(name="psB", bufs=2, space="PSUM"))
                emit_k_chunk(0, kvc_pre[0])
                psS0_cur = emit_s(0, 0)
                psS1_cur = emit_s(1, 0)
                emit_v_chunk(0, kvc_pre[0])
                for ci in range(NCI):
                    if ci + 1 < NCI:
                        if (ci + 1) % 4 == 0:
                            ch = (ci + 1) // 4
                            if ch + 1 < NCH:
                                kvc_pre.append(load_kv(ch + 1))
                            emit_k_chunk(ch, kvc_pre[ch])
                        psS0_nxt = emit_s(0, ci + 1)
                        psS1_nxt = emit_s(1, ci + 1)
                        if (ci + 1) % 4 == 0:
                            emit_v_chunk((ci + 1) // 4, kvc_pre[(ci + 1) // 4])
                    px0 = pexp.tile([128, Q], F16, name="px0", tag="px")
                    nc.scalar.activation(out=px0, in_=psS0_cur, func=EXP)
                    nc.scalar.activation(out=px1s[:, ci, :], in_=psS1_cur,
                                         func=EXP)
                    emit_o(po0, 0, ci, px0)
                    if ci + 1 < NCI:
                        psS0_cur = psS0_nxt
                        psS1_cur = psS1_nxt
                onT0 = emit_fin(po0, 0)

            # wo loaded between passes (SP queue idle here)
            wo_sb = wpool.tile([D, HPC, C], F16, name="wo_sb")
            nc.sync.dma_start(out=wo_sb, in_=wo[:, :, :])

            # ---- pass 2: h2 attention + h1's O + Wo(h0, h1) ----
            with contextlib.ExitStack() as pstk:
                psO2 = pstk.enter_context(
                    tc.tile_pool(name="psO2", bufs=1, space="PSUM"))
                po2 = psO2.tile([65, Q], F32, name="po2", tag="po")
                P['s'] = pstk.enter_context(
                    tc.tile_pool(name="psP2", bufs=2, space="PSUM"))
                psS2_cur = emit_s(2, 0)
                onT1 = None
                psF = None
                osb = None
                with contextlib.ExitStack() as pstk1:
                    psO1 = pstk1.enter_context(
                        tc.tile_pool(name="psO1", bufs=1, space="PSUM"))
                    po1 = psO1.tile([65, Q], F32, name="po1", tag="po")
                    for ci in range(NCI):
                        if ci + 1 < NCI:
                            psS2_nxt = emit_s(2, ci + 1)
                        px2 = pexp.tile([128, Q], F16, name="px2", tag="px")
                        nc.scalar.activation(out=px2, in_=psS2_cur, func=EXP)
                        emit_o(po2, 2, ci, px2)
                        if ci < 16:
                            emit_o(po1, 1, 2 * ci, px1s[:, 2 * ci, :])
                            emit_o(po1, 1, 2 * ci + 1, px1s[:, 2 * ci + 1, :])
                            if ci == 15:
                                onT1 = emit_fin(po1, 1)
                                pstk1.close()
                                psF = pstk.enter_context(
                                    tc.tile_pool(name="psF", bufs=2,
                                                 space="PSUM"))
                        else:
                            qt = ci - 16
                            hh, qtt = (0, qt) if qt < 8 else (1, qt - 8)
                            if qtt % 2 == 0:
                                osb = tailp.tile([128, 2, C], F16, name="osb",
                                                 tag="osb", bufs=4)
                            wo_half(psF, hh, onT0 if hh == 0 else onT1,
                                    qtt, osb)
                        if ci + 1 < NCI:
                            psS2_cur = psS2_nxt
                onT2 = emit_fin(po2, 2)
            with tc.tile_pool(name="psFt", bufs=3, space="PSUM") as psFt:
                for qt in range(8):
                    if qt % 2 == 0:
                        osb = tailp.tile([128, 2, C], F16, name="osb",
                                         tag="osb", bufs=4)
                    wo_half(psFt, 2, onT2, qt, osb, tail=True)
    nc.compile()
    return nc


def _get_compiled():
    global _COMPILED
    if _COMPILED is None:
        _COMPILED = _build()
    return _COMPILED


def _to_p128(a):
    """[768, M] -> [128, 6, M] partition-major fp16."""
    return np.ascontiguousarray(
        a.reshape(6, 128, -1).transpose(1, 0, 2)).astype(np.float16)


def _chunked(a, w):
    """[128, 6, M] -> [128, M//w, 6, w] (w-wide column chunks contiguous)."""
    p, t, m = a.shape
    return np.ascontiguousarray(
        a.reshape(p, t, m // w, w).transpose(0, 2, 1, 3))


def _make_in_maps(query, key_value, Wq, bq, Wk, bk, Wv, bv, Wo, rel_pos_bias):
    b1, w_n = _host_bias_parts(rel_pos_bias)
    scale = np.float32(SCALE)
    f16 = np.float16
    qTs = [_chunked(_to_p128(np.ascontiguousarray(query[b].T)), 512)
           for b in range(B)]
    kvTs = [_chunked(_to_p128(np.ascontiguousarray(key_value[b].T)), 512)
            for b in range(B)]
    w_n3 = np.ascontiguousarray(
        np.broadcast_to(w_n[:, None, :], (REL, HPC, N))).astype(f16)
    in_maps = []
    for c in range(N_CORES):
        b = c // (N_CORES // B)
        h0 = (c % (N_CORES // B)) * HPC
        cols = slice(D * h0, D * h0 + D * HPC)
        wkv = np.concatenate([Wk[:, cols], Wv[:, cols]], axis=1)
        bq6 = np.concatenate(
            [(bq[cols] * scale).reshape(HPC, D).T, bk[cols].reshape(HPC, D).T],
            axis=1)
        in_maps.append({
            "qT": qTs[b],
            "kvT": kvTs[b],
            "wqp": _to_p128(Wq[:, cols] * scale),
            "wkv": _to_p128(wkv),
            "wo": np.ascontiguousarray(
                Wo[cols, :].reshape(HPC, D, C).transpose(1, 0, 2)).astype(f16),
            "bq6": np.ascontiguousarray(bq6, dtype=np.float32),
            "bvb": np.ascontiguousarray(
                np.broadcast_to(bv[cols][None, :], (128, D * HPC))),
            "b1t": np.ascontiguousarray(
                b1[h0:h0 + HPC].transpose(2, 0, 1)).astype(f16),
            "w_n3": w_n3,
        })
    return in_maps


def kernel(query, key_value, Wq, bq, Wk, bk, Wv, bv, Wo, bo, rel_pos_bias):
    from concourse import bass_utils

    query = np.asarray(query, np.float32)
    key_value = np.asarray(key_value, np.float32)
    Wq = np.asarray(Wq, np.float32); bq = np.asarray(bq, np.float32)
    Wk = np.asarray(Wk, np.float32); bk = np.asarray(bk, np.float32)
    Wv = np.asarray(Wv, np.float32); bv = np.asarray(bv, np.float32)
    Wo = np.asarray(Wo, np.float32); bo = np.asarray(bo, np.float32)
    rel_pos_bias = np.asarray(rel_pos_bias, np.float32)

    in_maps = _make_in_maps(query, key_value, Wq, bq, Wk, bk, Wv, bv, Wo,
                            rel_pos_bias)
    nc = _get_compiled()
    res = bass_utils.run_bass_kernel_spmd(nc, in_maps,
                                          core_ids=list(range(N_CORES)))
    out = np.zeros((B, Q, C), np.float32)
    for c in range(N_CORES):
        b = c // (N_CORES // B)
        f = res.results[c]["out_p"].astype(np.float32)  # [HPC,Q,C] unnorm.
        d = res.results[c]["den"]            # [HPC, Q]
        out[b] += (f / d[:, :, None]).sum(axis=0)
    out += bo[None, None, :]
    return out
